# revision 15
# baseline (speedup 1.0000x reference)
"""Trainium2 Bass kernel for nn_Memory_30571577213131 (scatter_memory).

Slot-memory module: T=3 recurrence steps of {LayerNorm -> write-MHA(mem, z, z)
-> GRUCell} followed by a read-MHA(z, mem, mem).

Sharding: pure data parallel - batch B=64 split as 8 batches per core across
8 NeuronCores; all parameters replicated.

Key optimizations over a direct implementation:
  * All per-token (L=512) projections are folded onto the slot side (S=128)
    on the host (write K/V projections, read V/O projections, GRU input
    weights; softmax row-sum=1 absorbs value biases).
  * Step-0 specialization: at step 0 the memory is batch-invariant
    (broadcast slots), so the score fold M'0 = LN(slots) @ Wqk and the GRU
    hidden-side projections Whh_g @ LN(slots) are computed exactly on the
    HOST.  On-chip, the r/z hidden terms enter the gate PSUM via a single
    f16 matmul against a block-repeated identity (out[f,t] = gh0.T[t%128,f])
    and the n-gate hidden term is a direct SBUF operand.  This removes
    ~30% of the recurrence matmul work at zero accuracy cost.
  * fp8 (e4m3) DoubleRow matmuls (half the PE instruction count at FD=512)
    on the error-tolerant paths: write-attention score chain, the GRU
    input-side weights (gi) at all steps, the GRU hidden-side (gh) for the
    r gate at steps 1-2 and for all gates at step 1, and the read-attention
    score matmul (fold kept f16, quantized at AMP just before the token
    contraction).  Remaining hidden-side matmuls (z/n at step 2) stay f16.
  * The rank-1 write-score bias c_w @ z.T folds into the M' fold as a
    per-partition activation bias column (no separate matmuls).
  * LayerNorm split into stats (PE sums + row chain on vector/scalar) and
    apply (rstd broadcast + normalize), software-pipelined so the PE never
    stalls on the row chain.
  * Output written f16 and upcast on host.
"""

import numpy as np
import ml_dtypes
from contextlib import ExitStack

import concourse.bass as bass
import concourse.tile as tile
from concourse import bacc, mybir
from concourse import bass_utils
from concourse.masks import make_identity

f8 = mybir.dt.float8e4
f16 = mybir.dt.float16
f32 = mybir.dt.float32
AF = mybir.ActivationFunctionType
Alu = mybir.AluOpType
DR = mybir.MatmulPerfMode.DoubleRow

P = 128
E = 768
EC = E // P          # 6 feature chunks
S = 128              # slots
T = 3                # recurrence steps
B = 64
L = 512
NCORE = 8
NB = B // NCORE      # 8 batches per core
GB = 4               # batches per group (4*128 slots = 512 free dim)
NG = NB // GB        # 2 groups
LN_EPS = 1e-5

# fixed power-of-2 fp8 scales for activations
AZ = 16.0            # z
AU = 16.0            # U = A @ z
AM = 16.0            # memn (LN output)
AA = 128.0           # attention weights
AMP = 64.0           # M' = memn @ Wqk

# which gh (hidden-side) matmuls run fp8 per step (r, z, n); step 0 is host
FP8_GH = {1: ("r", "z", "n"), 2: ("r",)}
FP8_READ_SC = True   # read scores via fp8 M'' (fold stays f16)

# bias table column groups (6 wide) in the [128, NCOLS] f32 bias tile
LNG, LNB, SGR, SGZ, BGN, BHN, CR, CW = range(8)
NB6 = 8
COL_VC = NB6 * 6      # v_const single col
COL_SCN = NB6 * 6 + 1  # 1/(aWgn*AU) single col
NCOLS = NB6 * 6 + 2

_CACHE = {}


def _emit(nc, tc, ctx, D, sc):
    aWqk, aWgr, aWgz, aWgn = sc["aWqk"], sc["aWgr"], sc["aWgz"], sc["aWgn"]
    has_v, has_rbo = sc["has_v"], sc["has_rbo"]
    ln_affine = sc["ln_affine"]

    cp = ctx.enter_context(tc.tile_pool(name="consts", bufs=1))
    wp = ctx.enter_context(tc.tile_pool(name="wts", bufs=1))
    zp = ctx.enter_context(tc.tile_pool(name="zres", bufs=1))
    mnp = ctx.enter_context(tc.tile_pool(name="memn", bufs=3))
    mn8p = ctx.enter_context(tc.tile_pool(name="memn8", bufs=2))
    mp = ctx.enter_context(tc.tile_pool(name="mem", bufs=2))
    mpp = ctx.enter_context(tc.tile_pool(name="mpfold", bufs=2))
    utp = ctx.enter_context(tc.tile_pool(name="ut", bufs=2))
    gsp = ctx.enter_context(tc.tile_pool(name="gate_scratch", bufs=5))
    lsp = ctx.enter_context(tc.tile_pool(name="ln_scratch", bufs=4))
    sxp = ctx.enter_context(tc.tile_pool(name="softmax", bufs=2))
    anp = ctx.enter_context(tc.tile_pool(name="anpool", bufs=4))
    atp = ctx.enter_context(tc.tile_pool(name="att", bufs=4))
    rvp = ctx.enter_context(tc.tile_pool(name="readv", bufs=2))
    lnp = ctx.enter_context(tc.tile_pool(name="lnrows", bufs=1))
    smp = ctx.enter_context(tc.tile_pool(name="smalls", bufs=4))
    osp = ctx.enter_context(tc.tile_pool(name="outstage", bufs=4))
    psA = ctx.enter_context(tc.tile_pool(name="psA", bufs=4, space="PSUM"))
    psB = ctx.enter_context(tc.tile_pool(name="psB", bufs=2, space="PSUM"))
    psT = ctx.enter_context(tc.tile_pool(name="psT", bufs=2, space="PSUM"))

    # ---- constants
    idy16 = cp.tile([P, P], f16, tag="idy16")
    make_identity(nc, idy16[:])
    # identity repeated 4x along free dim: rhs for block-repeat matmuls
    idyrep = cp.tile([P, 512], f16, tag="idyrep")
    nc.vector.tensor_copy(idyrep[:, 0:128], idy16[:])
    nc.scalar.copy(idyrep[:, 128:256], idy16[:])
    nc.vector.tensor_copy(idyrep[:, 256:384], idy16[:])
    nc.gpsimd.tensor_copy(idyrep[:, 384:512], idy16[:])
    ones_c16 = cp.tile([P, 1], f16, tag="oc16")
    nc.vector.memset(ones_c16[:], 1.0)
    ones_r16 = cp.tile([1, P], f16, tag="or16")
    nc.vector.memset(ones_r16[:], 1.0)
    ones_r32 = cp.tile([1, P], mybir.dt.float32r, tag="or32")
    nc.scalar.copy(ones_r32[:], ones_r16[:])
    eps1 = cp.tile([1, 1], f32, tag="eps1")
    nc.vector.memset(eps1[:], LN_EPS)
    cAA = cp.tile([P, 1], f32, tag="cAA")
    nc.vector.memset(cAA[:], AA)
    cAM = cp.tile([P, 1], f32, tag="cAM")
    nc.vector.memset(cAM[:], AM)
    cinvE = cp.tile([1, 1], f32, tag="cinvE")
    nc.vector.memset(cinvE[:], 1.0 / E)
    cUsc = cp.tile([P, 1], f32, tag="cUsc")
    nc.vector.memset(cUsc[:], AU / (AA * AZ))
    cMPsc = cp.tile([P, 1], f32, tag="cMPsc")
    nc.vector.memset(cMPsc[:], AMP / (AM * aWqk))
    bias = cp.tile([P, NCOLS], f32, tag="bias")
    brep = cp.tile([1, E], f16, tag="brep") if has_rbo else None

    def bcol(i, c):
        return bias[:, i * 6 + c : i * 6 + c + 1]

    def b1col(i):
        return bias[:, i : i + 1]

    # ---- resident weights + z, DMA-ordered by first use
    # step-0 scores need mp0_8 + z8f; gates0 need wg8 + gh0/hn0 + z8t
    mp0_8 = wp.tile([P, EC, S], f8, tag="mp0_8")
    nc.gpsimd.dma_start(mp0_8[:], D["mp0_8"].rearrange("(c p) s -> p c s", p=P))
    nc.gpsimd.dma_start(bias[:], D["bias"])
    z8f = []
    for b in range(NB // 2):
        zf = zp.tile([P, EC, L], f8, tag=f"z8f{b}")
        nc.sync.dma_start(zf[:], D["z8f"][b].rearrange("(c p) t -> p c t", p=P))
        z8f.append(zf)
    wg8 = {}
    for gname in ("r", "z", "n"):
        w = wp.tile([P, EC, E], f8, tag=f"wg8{gname}")
        nc.scalar.dma_start(w[:], D[f"wg8{gname}"].rearrange("(c p) f -> p c f", p=P))
        wg8[gname] = w
    # gh0 terms (host-computed step-0 hidden projections)
    gh0t = {}
    for gname in ("r", "z"):
        g0 = wp.tile([P, E], f16, tag=f"gh0t{gname}")
        nc.gpsimd.dma_start(g0[:], D[f"gh0{gname}"])
        gh0t[gname] = g0
    hn0rep = mnp.tile([P, EC, 512], f16, tag="mn16")
    nc.gpsimd.dma_start(hn0rep[:], D["hn0"].rearrange("(c p) s -> p c s", p=P))
    z8t = []
    for b in range(NB):
        zt = zp.tile([P, 4, E], f8, tag=f"z8t{b}")
        nc.gpsimd.dma_start(zt[:], D["z8t"][b].rearrange("(t p) f -> p t f", p=P))
        z8t.append(zt)
    # initial memn (f16, for the GRU h elementwise term), shared by groups
    mn16_0 = mnp.tile([P, EC, 512], f16, tag="mn16")
    nc.gpsimd.dma_start(
        mn16_0[:, :, 0:128], D["mn0_16"].rearrange("(c p) s -> p c s", p=P))
    for bi in range(1, GB):
        nc.vector.tensor_copy(
            mn16_0[:, :, bi * 128 : (bi + 1) * 128], mn16_0[:, :, 0:128])
    for b in range(NB // 2, NB):
        zf = zp.tile([P, EC, L], f8, tag=f"z8f{b}")
        nc.sync.dma_start(zf[:], D["z8f"][b].rearrange("(c p) t -> p c t", p=P))
        z8f.append(zf)
    # step-1/2 weights
    wh8 = {}
    for gname in ("r", "z", "n"):
        w = wp.tile([P, EC, E], f8, tag=f"wh8{gname}")
        nc.scalar.dma_start(w[:], D[f"wh8{gname}"].rearrange("(c p) f -> p c f", p=P))
        wh8[gname] = w
    wqk8 = wp.tile([P, EC, E], f8, tag="wqk8")
    nc.scalar.dma_start(wqk8[:], D["wqk8"].rearrange("(c p) f -> p c f", p=P))
    # step-2 hidden weights + read weights share one 3-buffer tag: wvo's
    # buffer reuses wh16z's after the last step-2 z-gate matmul retires,
    # covered by the M'' fold matmuls that run first in the read phase.
    wh16 = {}
    for gname in ("z", "n"):
        w = wp.tile([P, EC, E], f16, tag="wbig", bufs=3)
        nc.scalar.dma_start(w[:], D[f"wh{gname}"].rearrange("(c p) f -> p c f", p=P))
        wh16[gname] = w
    wqkr = wp.tile([P, EC, E], f16, tag="wbig", bufs=3)
    nc.scalar.dma_start(wqkr[:], D["wqkr"].rearrange("(c p) f -> p c f", p=P))
    wvo = wp.tile([P, EC, E], f16, tag="wbig", bufs=3)
    nc.scalar.dma_start(wvo[:], D["wvo"].rearrange("(c p) f -> p c f", p=P))
    if has_v:
        wvcol = wp.tile([P, EC, 1], f16, tag="wvcol")
        nc.gpsimd.dma_start(wvcol[:], D["wvcol"].rearrange("(c p) f -> p c f", p=P))
    if has_rbo:
        nc.gpsimd.dma_start(brep[:], D["brep"])

    sig_r = 1.0 / (aWgr * AU)
    sig_z = 1.0 / (aWgz * AU)

    # ---- LayerNorm split: stats (PE sums + row chain) / apply (broadcast)
    def emit_ln_stats(src):
        psx = psB.tile([1, 512], f32, tag="psB")
        for e in range(EC):
            nc.tensor.matmul(
                psx[:], lhsT=ones_c16[:], rhs=src[:, e, :],
                start=(e == 0), stop=(e == EC - 1),
            )
        mur = lnp.tile([1, 512], f16, tag="mur")
        nc.vector.tensor_scalar_mul(mur[:], psx[:], cinvE[:])
        psq = psB.tile([1, 512], f32, tag="psB")
        for e in range(EC):
            sq = lsp.tile([P, 512], f16, tag="ls")
            if e % 2 == 0:
                nc.scalar.square(sq[:], src[:, e, :])
            else:
                nc.vector.tensor_mul(sq[:], src[:, e, :], src[:, e, :])
            nc.tensor.matmul(
                psq[:], lhsT=ones_c16[:], rhs=sq[:],
                start=(e == 0), stop=(e == EC - 1),
            )
        mu2 = lnp.tile([1, 512], f16, tag="mu2")
        nc.vector.tensor_mul(mu2[:], mur[:], mur[:])
        varr = lnp.tile([1, 512], f16, tag="e2r")
        nc.vector.scalar_tensor_tensor(
            varr[:], psq[:], 1.0 / E, mu2[:], op0=Alu.mult, op1=Alu.subtract
        )
        nc.scalar.activation(varr[:], varr[:], AF.Sqrt, bias=eps1[:])
        rir = lnp.tile([1, 512], mybir.dt.float32r, tag="rir")
        with nc.allow_low_precision(reason="f32r rstd row for broadcast matmul"):
            nc.vector.reciprocal(rir[:], varr[:])
        msr = lnp.tile([1, 512], f16, tag="msr")
        nc.vector.tensor_mul(msr[:], mur[:], rir[:])
        return rir, msr

    def emit_ln_apply(src, st):
        rir, msr = st
        m16 = mnp.tile([P, EC, 512], f16, tag="mn16")
        m8 = mn8p.tile([P, EC, 512], f8, tag="mn8")
        psb = psA.tile([P, 512], f32, tag="psA")
        nc.tensor.matmul(psb[:], lhsT=ones_r32[:], rhs=rir[:])
        rstdb = lsp.tile([P, 512], f16, tag="ls")
        nc.scalar.copy(rstdb[:], psb[:])
        psb2 = psA.tile([P, 512], f32, tag="psA")
        nc.tensor.matmul(psb2[:], lhsT=ones_r16[:], rhs=msr[:])
        msb = lsp.tile([P, 512], f16, tag="ls")
        nc.scalar.copy(msb[:], psb2[:])
        for e in range(EC):
            t1 = lsp.tile([P, 512], f16, tag="ls")
            nc.vector.tensor_mul(t1[:], src[:, e, :], rstdb[:])
            if ln_affine:
                t2 = lsp.tile([P, 512], f16, tag="ls")
                nc.vector.tensor_sub(t2[:], t1[:], msb[:])
                nc.vector.tensor_scalar(
                    m16[:, e, :], t2[:], bcol(LNG, e), bcol(LNB, e),
                    op0=Alu.mult, op1=Alu.add,
                )
            else:
                nc.vector.tensor_sub(m16[:, e, :], t1[:], msb[:])
            if e % 2 == 0:
                nc.scalar.activation(m8[:, e, :], m16[:, e, :], AF.Copy, scale=AM)
            else:
                nc.vector.tensor_scalar_mul(m8[:, e, :], m16[:, e, :], cAM[:])
        return m16, m8

    # ---- per-group phases
    def emit_fold(m8g):
        """M'8 = fp8( memn8 @ Wqk8 + c_w ), feature-major [P, EC, 512]."""
        mp8 = mpp.tile([P, EC, 512], f8, tag="mp8")
        for c in range(EC):
            ps = psA.tile([P, 512], f32, tag="psA")
            for ep in range(3):
                nc.tensor.matmul(
                    ps[:], lhsT=wqk8[:, 2 * ep : 2 * ep + 2, c * 128 : (c + 1) * 128],
                    rhs=m8g[:, 2 * ep : 2 * ep + 2, :],
                    start=(ep == 0), stop=(ep == 2), perf_mode=DR,
                )
            if c % 2 == 0:
                nc.vector.tensor_scalar(
                    mp8[:, c, :], ps[:], cMPsc[:], bcol(CW, c),
                    op0=Alu.mult, op1=Alu.add,
                )
            else:
                nc.scalar.activation(
                    mp8[:, c, :], ps[:], AF.Identity,
                    scale=AMP / (AM * aWqk), bias=bcol(CW, c))
        return mp8

    def emit_scores(g, mp8, step0=False):
        """scores+softmax for the 4 batches of group g -> normalized an list."""
        an8s = []
        for bi in range(GB):
            b = g * GB + bi
            ps = psA.tile([P, L], f32, tag="psA")
            for ep in range(3):
                if step0:
                    lt = mp8[:, 2 * ep : 2 * ep + 2, :]
                else:
                    lt = mp8[:, 2 * ep : 2 * ep + 2, bi * 128 : (bi + 1) * 128]
                nc.tensor.matmul(
                    ps[:], lhsT=lt, rhs=z8f[b][:, 2 * ep : 2 * ep + 2, :],
                    start=(ep == 0), stop=(ep == 2), perf_mode=DR,
                )
            aexp = sxp.tile([P, L], f16, tag="aexp")
            rsum = smp.tile([P, 1], f32, tag="rsum")
            nc.scalar.activation(
                aexp[:], ps[:], AF.Exp, scale=1.0 / (AMP * AZ), accum_out=rsum[:]
            )
            rinv = smp.tile([P, 1], f32, tag="rinv")
            nc.vector.reciprocal(rinv[:], rsum[:])
            an16 = anp.tile([P, L], f16, tag="an16")
            nc.vector.tensor_scalar_mul(an16[:], aexp[:], rinv[:])
            an8s.append(an16)
        return an8s

    def emit_transposes(an8s):
        att8 = []
        for bi in range(GB):
            at = atp.tile([P, 4, P], f8, tag="att8")
            pt4 = psT.tile([P, 4, P], f16, tag="psT")
            for t4 in range(4):
                nc.tensor.transpose(
                    pt4[:, t4, :], an8s[bi][:, t4 * 128 : (t4 + 1) * 128], idy16[:]
                )
            if bi % 2 == 0:
                nc.vector.tensor_scalar_mul(at[:], pt4[:], cAA[:])
            else:
                nc.scalar.activation(at[:], pt4[:], AF.Copy, scale=AA)
            att8.append(at)
        return att8

    def emit_ut(g, att8):
        """ut8 = fp8( (A@z).T ) feature-major [P, EC, 512]."""
        ut8 = utp.tile([P, EC, 512], f8, tag="ut8")
        for c in range(EC):
            ps = psA.tile([P, 512], f32, tag="psA")
            for bi in range(GB):
                b = g * GB + bi
                for tp in range(2):
                    nc.tensor.matmul(
                        ps[:, bi * 128 : (bi + 1) * 128],
                        lhsT=z8t[b][:, 2 * tp : 2 * tp + 2, c * 128 : (c + 1) * 128],
                        rhs=att8[bi][:, 2 * tp : 2 * tp + 2, :],
                        start=(tp == 0), stop=(tp == 1), perf_mode=DR,
                    )
            if c % 2 == 0:
                nc.vector.tensor_scalar_mul(ut8[:, c, :], ps[:], cUsc[:])
            else:
                nc.scalar.activation(
                    ut8[:, c, :], ps[:], AF.Copy, scale=AU / (AA * AZ)
                )
        return ut8

    def emit_gates(g, ut8, step, h16=None, chunks=None):
        """GRU gates chunk-major; writes h' to a new mem16 tile.

        step==0: hidden-side terms come from host (gh0t via block-repeat
        matmul for r/z, hn0rep as a direct SBUF operand for n).
        step>=1: hidden side contracts memn8 (fp8 DR) or memn16 (f16)
        per FP8_GH[step].
        """
        if h16 is None:
            h16 = mp.tile([P, EC, 512], f16, tag="h16")
        fp8g = FP8_GH.get(step, ())
        mn16g = mn16_0 if step == 0 else memn16[g]
        for c in (chunks if chunks is not None else range(EC)):
            cs = slice(c * 128, (c + 1) * 128)

            def gi_mms(psum, gname, stop_last):
                for ep in range(3):
                    nc.tensor.matmul(
                        psum[:], lhsT=wg8[gname][:, 2 * ep : 2 * ep + 2, cs],
                        rhs=ut8[:, 2 * ep : 2 * ep + 2, :],
                        start=(ep == 0), stop=(stop_last and ep == 2),
                        perf_mode=DR,
                    )

            def gh_mms(psum, gname):
                # hidden-side contraction for steps >= 1
                if gname in fp8g:
                    for ep in range(3):
                        nc.tensor.matmul(
                            psum[:], lhsT=wh8[gname][:, 2 * ep : 2 * ep + 2, cs],
                            rhs=memn8[g][:, 2 * ep : 2 * ep + 2, :],
                            start=False, stop=(ep == 2), perf_mode=DR,
                        )
                else:
                    for e in range(EC):
                        nc.tensor.matmul(
                            psum[:], lhsT=wh16[gname][:, e, cs],
                            rhs=mn16g[:, e, :],
                            start=False, stop=(e == EC - 1),
                        )

            ps_r = psA.tile([P, 512], f32, tag="psA")
            gi_mms(ps_r, "r", stop_last=False)
            if step == 0:
                nc.tensor.matmul(
                    ps_r[:], lhsT=gh0t["r"][:, cs], rhs=idyrep[:],
                    start=False, stop=True,
                )
            else:
                gh_mms(ps_r, "r")
            r16 = gsp.tile([P, 512], f16, tag="gs")
            nc.scalar.activation(
                r16[:], ps_r[:], AF.Sigmoid, scale=sig_r, bias=bcol(SGR, c)
            )

            ps_z = psA.tile([P, 512], f32, tag="psA")
            gi_mms(ps_z, "z", stop_last=False)
            if step == 0:
                nc.tensor.matmul(
                    ps_z[:], lhsT=gh0t["z"][:, cs], rhs=idyrep[:],
                    start=False, stop=True,
                )
            else:
                gh_mms(ps_z, "z")
            zt16 = gsp.tile([P, 512], f16, tag="gs")
            nc.scalar.activation(
                zt16[:], ps_z[:], AF.Sigmoid, scale=sig_z, bias=bcol(SGZ, c)
            )

            ps_i = psA.tile([P, 512], f32, tag="psA")
            gi_mms(ps_i, "n", stop_last=True)
            if step == 0:
                hn16 = hn0rep[:, c, :]
            else:
                ps_h = psA.tile([P, 512], f32, tag="psA")
                if "n" in fp8g:
                    for ep in range(3):
                        nc.tensor.matmul(
                            ps_h[:], lhsT=wh8["n"][:, 2 * ep : 2 * ep + 2, cs],
                            rhs=memn8[g][:, 2 * ep : 2 * ep + 2, :],
                            start=(ep == 0), stop=(ep == 2), perf_mode=DR,
                        )
                else:
                    for e in range(EC):
                        nc.tensor.matmul(
                            ps_h[:], lhsT=wh16["n"][:, e, cs], rhs=mn16g[:, e, :],
                            start=(e == 0), stop=(e == EC - 1),
                        )
                hn16 = gsp.tile([P, 512], f16, tag="gs")
                nc.vector.tensor_scalar(
                    hn16[:], ps_h[:], b1col(COL_SCN), bcol(BHN, c),
                    op0=Alu.mult, op1=Alu.add,
                )
            t16 = gsp.tile([P, 512], f16, tag="gs")
            nc.vector.tensor_mul(t16[:], r16[:], hn16[:])
            # arg = gi_n/(aWgn*AU) + r*hn; the +bgn bias rides the tanh
            arg16 = gsp.tile([P, 512], f16, tag="gs")
            nc.vector.scalar_tensor_tensor(
                arg16[:], ps_i[:], b1col(COL_SCN), t16[:],
                op0=Alu.mult, op1=Alu.add,
            )
            n16 = gsp.tile([P, 512], f16, tag="gs")
            nc.scalar.activation(n16[:], arg16[:], AF.Tanh, bias=bcol(BGN, c))
            d16 = gsp.tile([P, 512], f16, tag="gs")
            nc.vector.tensor_sub(d16[:], mn16g[:, c, :], n16[:])
            t2 = gsp.tile([P, 512], f16, tag="gs")
            nc.vector.tensor_mul(t2[:], zt16[:], d16[:])
            nc.vector.tensor_add(h16[:, c, :], n16[:], t2[:])
        return h16

    # ---- step 0 (batch-invariant memory; fold + hidden side from host)
    memn16 = [None, None]
    memn8 = [None, None]
    an0 = emit_scores(0, mp0_8, step0=True)
    att0 = emit_transposes(an0)
    ut0 = emit_ut(0, att0)
    an1 = emit_scores(1, mp0_8, step0=True)
    att1 = emit_transposes(an1)
    h0 = emit_gates(0, ut0, 0, chunks=range(0, 2))
    ut1 = emit_ut(1, att1)
    h0 = emit_gates(0, ut0, 0, h16=h0, chunks=range(2, EC))
    s0 = emit_ln_stats(h0)
    h1 = emit_gates(1, ut1, 0, chunks=range(0, 3))
    memn16[0], memn8[0] = emit_ln_apply(h0, s0)
    h1 = emit_gates(1, ut1, 0, h16=h1, chunks=range(3, EC))
    pend1 = (h1, emit_ln_stats(h1))

    # ---- steps 1..T-1, LN-apply for group 1 deferred into the next step
    for step in range(1, T):
        mp8_0 = emit_fold(memn8[0])
        an0 = emit_scores(0, mp8_0)
        att0 = emit_transposes(an0)
        if pend1 is not None:
            hp, sp = pend1
            memn16[1], memn8[1] = emit_ln_apply(hp, sp)
            pend1 = None
        ut0 = emit_ut(0, att0)
        mp8_1 = emit_fold(memn8[1])
        an1 = emit_scores(1, mp8_1)
        att1 = emit_transposes(an1)
        h0 = emit_gates(0, ut0, step, chunks=range(0, 2))
        ut1 = emit_ut(1, att1)
        h0 = emit_gates(0, ut0, step, h16=h0, chunks=range(2, EC))
        if step < T - 1:
            s0 = emit_ln_stats(h0)
            h1 = emit_gates(1, ut1, step, chunks=range(0, 3))
            memn16[0], memn8[0] = emit_ln_apply(h0, s0)
            h1 = emit_gates(1, ut1, step, h16=h1, chunks=range(3, EC))
            pend1 = (h1, emit_ln_stats(h1))
        else:
            h1 = emit_gates(1, ut1, step)
            mem16 = [h0, h1]

    # ---- read attention: M'' (fm, fp8 at AMP), v, V'' (slot-major) per group
    m28_g, v16_g, vpp16_g = [], [], []
    for g in range(NG):
        m28 = mpp.tile([P, EC, 512], f8, tag="mp8")
        for c in range(EC):
            ps = psA.tile([P, 512], f32, tag="psA")
            for e in range(EC):
                nc.tensor.matmul(
                    ps[:], lhsT=wqkr[:, e, c * 128 : (c + 1) * 128],
                    rhs=mem16[g][:, e, :],
                    start=(e == 0), stop=(e == EC - 1),
                )
            if c % 2 == 0:
                nc.vector.tensor_scalar(
                    m28[:, c, :], ps[:], AMP, bcol(CR, c),
                    op0=Alu.mult, op1=Alu.add,
                )
            else:
                nc.scalar.activation(
                    m28[:, c, :], ps[:], AF.Identity, scale=AMP, bias=bcol(CR, c))
        m28_g.append(m28)
        if has_v:
            v16 = rvp.tile([P, GB], f32, tag="v16")
            for bi in range(GB):
                psv = psT.tile([P, 1], f32, tag="psT")
                for e in range(EC):
                    nc.tensor.matmul(
                        psv[:], lhsT=mem16[g][:, e, bi * 128 : (bi + 1) * 128],
                        rhs=wvcol[:, e, :],
                        start=(e == 0), stop=(e == EC - 1),
                    )
                nc.scalar.activation(
                    v16[:, bi : bi + 1], psv[:], AF.Identity, bias=b1col(COL_VC)
                )
            v16_g.append(v16)
        else:
            v16_g.append(None)
        vpp = rvp.tile([P, GB, E], f16, tag="vpp16")
        for bi in range(GB):
            for n0, nw in ((0, 512), (512, 256)):
                ps = psA.tile([P, nw], f32, tag="psA")
                for e in range(EC):
                    nc.tensor.matmul(
                        ps[:], lhsT=mem16[g][:, e, bi * 128 : (bi + 1) * 128],
                        rhs=wvo[:, e, n0 : n0 + nw],
                        start=(e == 0), stop=(not has_rbo and e == EC - 1),
                    )
                # rank-1 + 1 x rbo_eff: after the deferred 1/colsum scaling the
                # per-token bias comes out exactly as rbo_eff.
                if has_rbo:
                    nc.tensor.matmul(
                        ps[:], lhsT=ones_r16[:], rhs=brep[:, n0 : n0 + nw],
                        start=False, stop=True,
                    )
                if bi % 2 == 0:
                    nc.scalar.copy(vpp[:, bi, n0 : n0 + nw], ps[:])
                else:
                    nc.vector.tensor_copy(vpp[:, bi, n0 : n0 + nw], ps[:])
        vpp16_g.append(vpp)

    # ---- read attention per batch, software-pipelined by one batch so the
    # PE's AV matmuls of batch i overlap the exp/recip of batch i+1.
    def emit_read_scores(idx):
        g, bi = idx // GB, idx % GB
        b = idx
        ps_s = psA.tile([P, L], f32, tag="psA")
        if FP8_READ_SC:
            for ep in range(3):
                nc.tensor.matmul(
                    ps_s[:],
                    lhsT=m28_g[g][:, 2 * ep : 2 * ep + 2, bi * 128 : (bi + 1) * 128],
                    rhs=z8f[b][:, 2 * ep : 2 * ep + 2, :],
                    start=(ep == 0), stop=(ep == 2), perf_mode=DR,
                )
        else:
            for e in range(EC):
                nc.tensor.matmul(
                    ps_s[:],
                    lhsT=m28_g[g][:, e, bi * 128 : (bi + 1) * 128],
                    rhs=z8f[b][:, e, :],
                    start=(e == 0), stop=(e == EC - 1),
                )
        eS = sxp.tile([P, L], f16, tag="eS")
        if has_v:
            nc.scalar.activation(
                eS[:], ps_s[:], AF.Exp, scale=1.0 / (AMP * AZ),
                bias=v16_g[g][:, bi : bi + 1],
            )
        else:
            nc.scalar.activation(eS[:], ps_s[:], AF.Exp, scale=1.0 / (AMP * AZ))
        return eS

    def emit_read_av(idx, eS):
        g, bi = idx // GB, idx % GB
        b = idx
        rc4ps = psT.tile([P, 4], f32, tag="psT")
        for t4 in range(4):
            nc.tensor.matmul(
                rc4ps[:, t4 : t4 + 1],
                lhsT=eS[:, t4 * 128 : (t4 + 1) * 128],
                rhs=ones_c16[:],
            )
        rc4 = smp.tile([P, 4], f32, tag="rc4")
        nc.vector.reciprocal(rc4[:], rc4ps[:])
        for t4 in range(4):
            osb = osp.tile([P, E], f16, tag="osb")
            for n0, nw in ((0, 512), (512, 256)):
                if t4 % 2 == 0:
                    ps = psB.tile([P, nw], f32, tag="psB")
                else:
                    ps = psA.tile([P, nw], f32, tag="psA")
                nc.tensor.matmul(
                    ps[:],
                    lhsT=eS[:, t4 * 128 : (t4 + 1) * 128],
                    rhs=vpp16_g[g][:, bi, n0 : n0 + nw],
                )
                if n0 == 0:
                    nc.vector.tensor_scalar_mul(
                        osb[:, n0 : n0 + nw], ps[:], rc4[:, t4 : t4 + 1]
                    )
                else:
                    nc.scalar.activation(
                        osb[:, n0 : n0 + nw], ps[:], AF.Copy,
                        scale=rc4[:, t4 : t4 + 1],
                    )
            if t4 % 2 == 0:
                nc.sync.dma_start(D["out"][b, t4 * 128 : (t4 + 1) * 128, :], osb[:])
            else:
                nc.scalar.dma_start(D["out"][b, t4 * 128 : (t4 + 1) * 128, :], osb[:])

    eS_prev = None
    for idx in range(NB):
        eS_cur = emit_read_scores(idx)
        if eS_prev is not None:
            emit_read_av(idx - 1, eS_prev)
        eS_prev = eS_cur
    emit_read_av(NB - 1, eS_prev)


def _build(sc):
    key = tuple(sorted(sc.items()))
    if key in _CACHE:
        return _CACHE[key]
    nc = bacc.Bacc(
        "TRN2", target_bir_lowering=False, debug=False, enable_asserts=False
    )
    D = {}
    D["z8f"] = nc.dram_tensor("z8f", [NB, E, L], f8, kind="ExternalInput").ap()
    D["z8t"] = nc.dram_tensor("z8t", [NB, L, E], f8, kind="ExternalInput").ap()
    D["wqk8"] = nc.dram_tensor("wqk8", [E, E], f8, kind="ExternalInput").ap()
    for gname in ("r", "z", "n"):
        D[f"wg8{gname}"] = nc.dram_tensor(
            f"wg8{gname}", [E, E], f8, kind="ExternalInput").ap()
        D[f"wh8{gname}"] = nc.dram_tensor(
            f"wh8{gname}", [E, E], f8, kind="ExternalInput").ap()
    for gname in ("z", "n"):
        D[f"wh{gname}"] = nc.dram_tensor(
            f"wh{gname}", [E, E], f16, kind="ExternalInput").ap()
    D["wqkr"] = nc.dram_tensor("wqkr", [E, E], f16, kind="ExternalInput").ap()
    D["wvo"] = nc.dram_tensor("wvo", [E, E], f16, kind="ExternalInput").ap()
    D["wvcol"] = nc.dram_tensor("wvcol", [E, 1], f16, kind="ExternalInput").ap()
    D["mn0_16"] = nc.dram_tensor("mn0_16", [E, S], f16, kind="ExternalInput").ap()
    D["mp0_8"] = nc.dram_tensor("mp0_8", [E, S], f8, kind="ExternalInput").ap()
    D["gh0r"] = nc.dram_tensor("gh0r", [S, E], f16, kind="ExternalInput").ap()
    D["gh0z"] = nc.dram_tensor("gh0z", [S, E], f16, kind="ExternalInput").ap()
    D["hn0"] = nc.dram_tensor("hn0", [E, 512], f16, kind="ExternalInput").ap()
    D["bias"] = nc.dram_tensor("bias", [P, NCOLS], f32, kind="ExternalInput").ap()
    D["brep"] = nc.dram_tensor("brep", [1, E], f16, kind="ExternalInput").ap()
    D["out"] = nc.dram_tensor("out", [NB, L, E], f16, kind="ExternalOutput").ap()
    with tile.TileContext(nc) as tc:
        with ExitStack() as ctx:
            _emit(nc, tc, ctx, D, sc)
    nc.compile()
    _CACHE[key] = nc
    return nc


def _pow2_alpha(x, target=120.0):
    m = float(np.abs(x).max())
    if m < 1e-30:
        return 1.0
    return float(2.0 ** np.floor(np.log2(target / m)))


def _q8(x, alpha):
    xs = np.asarray(x, np.float32) * alpha
    return np.ascontiguousarray(
        np.clip(xs, -224.0, 224.0).astype(ml_dtypes.float8_e4m3)
    )


def _t16(a):
    return np.ascontiguousarray(np.asarray(a, np.float32).astype(np.float16))


def _host_prep(inp):
    s = 1.0 / np.sqrt(float(E))
    W = {k: np.asarray(v, np.float64) for k, v in inp.items()}

    # --- folds (f64)
    Wqk_w = (W["w_wq"].T @ W["w_wk"]) * s
    c_w = (W["w_bq"] @ W["w_wk"]) * s
    wo, wv = W["w_wo"], W["w_wv"]
    wih, whh = W["gru_wih"], W["gru_whh"]
    bih, bhh = W["gru_bih"], W["gru_bhh"]
    bo, bv = W["w_bo"], W["w_bv"]
    Wg, bgc, Whg, bhgc = {}, {}, {}, {}
    for i, gname in enumerate(("r", "z", "n")):
        wg = wih[i * E:(i + 1) * E]
        Wg[gname] = wg @ wo @ wv
        bgc[gname] = bv @ (wg @ wo).T + bo @ wg.T + bih[i * E:(i + 1) * E]
        Whg[gname] = whh[i * E:(i + 1) * E]
        bhgc[gname] = bhh[i * E:(i + 1) * E]
    Wqk_r = (W["r_wk"].T @ W["r_wq"]) * s
    c_r = (W["r_bk"] @ W["r_wq"]) * s
    w_v = (W["r_wk"].T @ W["r_bq"]) * s
    v_const = float((W["r_bk"] @ W["r_bq"]) * s)
    Wvo_r = W["r_wv"].T @ W["r_wo"].T
    c_vo = W["r_bv"] @ W["r_wo"].T
    rbo_eff = W["r_bo"] + c_vo

    # --- scales
    sc = {
        "ln_affine": bool(
            np.abs(np.asarray(W["ln_g"]) - 1.0).max() > 0
            or np.abs(np.asarray(W["ln_b"])).max() > 0
        ),
        "has_v": bool(np.abs(w_v).max() > 0 or v_const != 0.0),
        "has_rbo": bool(np.abs(rbo_eff).max() > 0),
        "aWqk": _pow2_alpha(Wqk_w),
        "aWgr": _pow2_alpha(np.concatenate([Wg["r"].ravel(), Whg["r"].ravel()])),
        "aWgz": _pow2_alpha(np.concatenate([Wg["z"].ravel(), Whg["z"].ravel()])),
        "aWgn": _pow2_alpha(np.concatenate([Wg["n"].ravel(), Whg["n"].ravel()])),
    }
    for gname, a in (("r", sc["aWgr"]), ("z", sc["aWgz"]), ("n", sc["aWgn"])):
        assert np.abs(Whg[gname]).max() * a * AU < 60000.0

    shared = {}
    # weights stored [in, out] so lhsT tiles contract over the partition dim
    shared["wqk8"] = _q8(Wqk_w, sc["aWqk"])
    for gname in ("r", "z", "n"):
        a = sc[f"aWg{gname}"]
        shared[f"wg8{gname}"] = _q8(Wg[gname].T, a)     # [in, out]
        shared[f"wh8{gname}"] = _q8(Whg[gname].T, a)    # [in, out] fp8
    for gname in ("z", "n"):
        a = sc[f"aWg{gname}"]
        shared[f"wh{gname}"] = _t16(Whg[gname].T * (a * AU))
    shared["wqkr"] = _t16(Wqk_r)
    shared["wvo"] = _t16(Wvo_r)
    shared["wvcol"] = _t16(w_v.reshape(E, 1))

    # initial memn = LN(slots) (batch-invariant), feature-major [E, S]
    slots = np.asarray(W["slots"], np.float64)[0]       # [S, E]
    g64 = W["ln_g"]
    b64 = W["ln_b"]
    mu = slots.mean(-1, keepdims=True)
    var = ((slots - mu) ** 2).mean(-1, keepdims=True)
    mn0 = (slots - mu) / np.sqrt(var + LN_EPS) * g64 + b64   # [S, E]
    shared["mn0_16"] = _t16(mn0.T)

    # step-0 host precomputes: score fold and GRU hidden-side projections
    shared["mp0_8"] = _q8((mn0 @ Wqk_w + c_w[None, :]).T, AMP)     # [E, S]
    for gname in ("r", "z"):
        a = sc[f"aWg{gname}"]
        g0 = (mn0 @ Whg[gname].T) * (a * AU)                       # [S, E]
        assert np.abs(g0).max() < 60000.0
        shared[f"gh0{gname}"] = _t16(g0)
    hn0 = (mn0 @ Whg["n"].T) + bhgc["n"][None, :]                  # [S, E]
    assert np.abs(hn0).max() < 60000.0
    shared["hn0"] = np.ascontiguousarray(np.tile(_t16(hn0).T, (1, GB)))  # [E,512]

    def col6(vec):
        return np.asarray(vec, np.float32).reshape(EC, P).T

    cols = [
        col6(W["ln_g"]), col6(W["ln_b"]),
        col6(bgc["r"] + bhgc["r"]), col6(bgc["z"] + bhgc["z"]),
        col6(bgc["n"]), col6(bhgc["n"]),
        col6(c_r * AMP), col6(c_w * AMP),
    ]
    btab = np.concatenate(cols, axis=1)
    vc = np.full((P, 1), v_const, np.float32)
    scn = np.full((P, 1), 1.0 / (sc["aWgn"] * AU), np.float32)
    shared["bias"] = np.ascontiguousarray(
        np.concatenate([btab, vc, scn], axis=1), np.float32)
    shared["brep"] = np.ascontiguousarray(
        np.asarray(rbo_eff, np.float32)[None, :].astype(np.float16))

    # --- per-core z arrays
    z = np.asarray(inp["z"], np.float32)                # [B, L, E]
    zT = np.swapaxes(z, 1, 2)                           # [B, E, L]
    z8f_all = _q8(zT, AZ)
    z8t_all = _q8(z, AZ)
    in_maps = []
    for c in range(NCORE):
        m = dict(shared)
        sl = slice(c * NB, (c + 1) * NB)
        m["z8f"] = np.ascontiguousarray(z8f_all[sl])
        m["z8t"] = np.ascontiguousarray(z8t_all[sl])
        in_maps.append(m)
    return in_maps, sc


def kernel(**inputs):
    in_maps, sc = _host_prep(inputs)
    nc = _build(sc)
    res = bass_utils.run_bass_kernel_spmd(nc, in_maps, core_ids=list(range(NCORE)))
    out = np.concatenate([res.results[c]["out"] for c in range(NCORE)], axis=0)
    return out.astype(np.float32)


# revision 16
# speedup vs baseline: 1.0610x; 1.0610x over previous
"""Trainium2 Bass kernel for nn_Memory_30571577213131 (scatter_memory).

Slot-memory module: T=3 recurrence steps of {LayerNorm -> write-MHA(mem, z, z)
-> GRUCell} followed by a read-MHA(z, mem, mem).

Sharding: pure data parallel - batch B=64 split as 8 batches per core across
8 NeuronCores; all parameters replicated.

Key optimizations over a direct implementation:
  * All per-token (L=512) projections are folded onto the slot side (S=128)
    on the host (write K/V projections, read V/O projections, GRU input
    weights; softmax row-sum=1 absorbs value biases).
  * Step-0 specialization: at step 0 the memory is batch-invariant
    (broadcast slots), so the score fold M'0 = LN(slots) @ Wqk and the GRU
    hidden-side projections Whh_g @ LN(slots) are computed exactly on the
    HOST.  On-chip, the r/z hidden terms enter the gate PSUM via a single
    f16 matmul against a block-repeated identity (out[f,t] = gh0.T[t%128,f])
    and the n-gate hidden term is a direct SBUF operand.  This removes
    ~30% of the recurrence matmul work at zero accuracy cost.
  * fp8 (e4m3) DoubleRow matmuls (half the PE instruction count at FD=512)
    on the error-tolerant paths: write-attention score chain, the GRU
    input-side weights (gi) at all steps, the GRU hidden-side (gh) for the
    r gate at steps 1-2 and for all gates at step 1, and the read-attention
    score matmul (fold kept f16, quantized at AMP just before the token
    contraction).  Remaining hidden-side matmuls (z/n at step 2) stay f16.
  * The rank-1 write-score bias c_w @ z.T folds into the M' fold as a
    per-partition activation bias column (no separate matmuls).
  * LayerNorm split into stats (PE sums + row chain on vector/scalar) and
    apply (rstd broadcast + normalize), software-pipelined so the PE never
    stalls on the row chain.
  * Output written f16 and upcast on host.
"""

import numpy as np
import ml_dtypes
from contextlib import ExitStack

import concourse.bass as bass
import concourse.tile as tile
from concourse import bacc, mybir
from concourse import bass_utils
from concourse.masks import make_identity

f8 = mybir.dt.float8e4
f16 = mybir.dt.float16
f32 = mybir.dt.float32
AF = mybir.ActivationFunctionType
Alu = mybir.AluOpType
DR = mybir.MatmulPerfMode.DoubleRow

P = 128
E = 768
EC = E // P          # 6 feature chunks
S = 128              # slots
T = 3                # recurrence steps
B = 64
L = 512
NCORE = 8
NB = B // NCORE      # 8 batches per core
GB = 4               # batches per group (4*128 slots = 512 free dim)
NG = NB // GB        # 2 groups
LN_EPS = 1e-5

# fixed power-of-2 fp8 scales for activations
AZ = 16.0            # z
AU = 16.0            # U = A @ z
AM = 16.0            # memn (LN output)
AA = 128.0           # attention weights
AMP = 64.0           # M' = memn @ Wqk

# which gh (hidden-side) matmuls run fp8 per step (r, z, n); step 0 is host
FP8_GH = {1: ("r", "z", "n"), 2: ("r",)}
FP8_READ_SC = True   # read scores via fp8 M'' (fold stays f16)

# bias table column groups (6 wide) in the [128, NCOLS] f32 bias tile
LNG, LNB, SGR, SGZ, BGN, BHN, CR, CW = range(8)
NB6 = 8
COL_VC = NB6 * 6      # v_const single col
COL_SCN = NB6 * 6 + 1  # 1/(aWgn*AU) single col
NCOLS = NB6 * 6 + 2

_CACHE = {}


def _emit(nc, tc, ctx, D, sc):
    aWqk, aWgr, aWgz, aWgn = sc["aWqk"], sc["aWgr"], sc["aWgz"], sc["aWgn"]
    has_v, has_rbo = sc["has_v"], sc["has_rbo"]
    ln_affine = sc["ln_affine"]

    cp = ctx.enter_context(tc.tile_pool(name="consts", bufs=1))
    wp = ctx.enter_context(tc.tile_pool(name="wts", bufs=1))
    zp = ctx.enter_context(tc.tile_pool(name="zres", bufs=1))
    mnp = ctx.enter_context(tc.tile_pool(name="memn", bufs=3))
    mn8p = ctx.enter_context(tc.tile_pool(name="memn8", bufs=2))
    mp = ctx.enter_context(tc.tile_pool(name="mem", bufs=2))
    mpp = ctx.enter_context(tc.tile_pool(name="mpfold", bufs=2))
    utp = ctx.enter_context(tc.tile_pool(name="ut", bufs=2))
    gsp = ctx.enter_context(tc.tile_pool(name="gate_scratch", bufs=5))
    lsp = ctx.enter_context(tc.tile_pool(name="ln_scratch", bufs=4))
    sxp = ctx.enter_context(tc.tile_pool(name="softmax", bufs=2))
    anp = ctx.enter_context(tc.tile_pool(name="anpool", bufs=4))
    atp = ctx.enter_context(tc.tile_pool(name="att", bufs=4))
    rvp = ctx.enter_context(tc.tile_pool(name="readv", bufs=2))
    lnp = ctx.enter_context(tc.tile_pool(name="lnrows", bufs=1))
    smp = ctx.enter_context(tc.tile_pool(name="smalls", bufs=4))
    osp = ctx.enter_context(tc.tile_pool(name="outstage", bufs=4))
    psA = ctx.enter_context(tc.tile_pool(name="psA", bufs=4, space="PSUM"))
    psB = ctx.enter_context(tc.tile_pool(name="psB", bufs=2, space="PSUM"))
    psT = ctx.enter_context(tc.tile_pool(name="psT", bufs=2, space="PSUM"))

    # ---- constants
    idy16 = cp.tile([P, P], f16, tag="idy16")
    make_identity(nc, idy16[:])
    # identity repeated 4x along free dim: rhs for block-repeat matmuls
    idyrep = cp.tile([P, 512], f16, tag="idyrep")
    nc.vector.tensor_copy(idyrep[:, 0:128], idy16[:])
    nc.scalar.copy(idyrep[:, 128:256], idy16[:])
    nc.vector.tensor_copy(idyrep[:, 256:384], idy16[:])
    nc.gpsimd.tensor_copy(idyrep[:, 384:512], idy16[:])
    ones_c16 = cp.tile([P, 1], f16, tag="oc16")
    nc.vector.memset(ones_c16[:], 1.0)
    ones_r16 = cp.tile([1, P], f16, tag="or16")
    nc.vector.memset(ones_r16[:], 1.0)
    ones_r32 = cp.tile([1, P], mybir.dt.float32r, tag="or32")
    nc.scalar.copy(ones_r32[:], ones_r16[:])
    eps1 = cp.tile([1, 1], f32, tag="eps1")
    nc.vector.memset(eps1[:], LN_EPS)
    cAA = cp.tile([P, 1], f32, tag="cAA")
    nc.vector.memset(cAA[:], AA)
    cAM = cp.tile([P, 1], f32, tag="cAM")
    nc.vector.memset(cAM[:], AM)
    cinvE = cp.tile([1, 1], f32, tag="cinvE")
    nc.vector.memset(cinvE[:], 1.0 / E)
    cUsc = cp.tile([P, 1], f32, tag="cUsc")
    nc.vector.memset(cUsc[:], AU / (AA * AZ))
    cMPsc = cp.tile([P, 1], f32, tag="cMPsc")
    nc.vector.memset(cMPsc[:], AMP / (AM * aWqk))
    bias = cp.tile([P, NCOLS], f32, tag="bias")
    brep = cp.tile([1, E], f16, tag="brep") if has_rbo else None

    def bcol(i, c):
        return bias[:, i * 6 + c : i * 6 + c + 1]

    def b1col(i):
        return bias[:, i : i + 1]

    # ---- resident weights + z, DMA-ordered by first use
    # step-0 scores need mp0_8 + z8f; gates0 need wg8 + gh0/hn0 + z8t
    mp0_8 = wp.tile([P, EC, S], f8, tag="mp0_8")
    nc.sync.dma_start(mp0_8[:], D["mp0_8"].rearrange("(c p) s -> p c s", p=P))
    nc.sync.dma_start(bias[:], D["bias"])
    z8f = []
    for b in range(NB // 2):
        zf = zp.tile([P, EC, L], f8, tag=f"z8f{b}")
        nc.sync.dma_start(zf[:], D["z8f"][b].rearrange("(c p) t -> p c t", p=P))
        z8f.append(zf)
    wg8 = {}
    for gname in ("r", "z", "n"):
        w = wp.tile([P, EC, E], f8, tag=f"wg8{gname}")
        nc.sync.dma_start(w[:], D[f"wg8{gname}"].rearrange("(c p) f -> p c f", p=P))
        wg8[gname] = w
    # gh0 terms (host-computed step-0 hidden projections)
    gh0t = {}
    for gname in ("r", "z"):
        g0 = wp.tile([P, E], f16, tag=f"gh0t{gname}")
        nc.sync.dma_start(g0[:], D[f"gh0{gname}"])
        gh0t[gname] = g0
    hn0rep = mnp.tile([P, EC, 512], f16, tag="mn16")
    nc.sync.dma_start(hn0rep[:], D["hn0"].rearrange("(c p) s -> p c s", p=P))
    z8t = []
    for b in range(NB // 2):
        zt = zp.tile([P, 4, E], f8, tag=f"z8t{b}")
        nc.sync.dma_start(zt[:], D["z8t"][b].rearrange("(t p) f -> p t f", p=P))
        z8t.append(zt)
    # initial memn (f16, for the GRU h elementwise term), shared by groups
    mn16_0 = mnp.tile([P, EC, 512], f16, tag="mn16")
    nc.sync.dma_start(
        mn16_0[:, :, 0:128], D["mn0_16"].rearrange("(c p) s -> p c s", p=P))
    for bi in range(1, GB):
        nc.vector.tensor_copy(
            mn16_0[:, :, bi * 128 : (bi + 1) * 128], mn16_0[:, :, 0:128])
    for b in range(NB // 2, NB):
        zf = zp.tile([P, EC, L], f8, tag=f"z8f{b}")
        nc.sync.dma_start(zf[:], D["z8f"][b].rearrange("(c p) t -> p c t", p=P))
        z8f.append(zf)
    for b in range(NB // 2, NB):
        zt = zp.tile([P, 4, E], f8, tag=f"z8t{b}")
        nc.sync.dma_start(zt[:], D["z8t"][b].rearrange("(t p) f -> p t f", p=P))
        z8t.append(zt)
    # step-1/2 weights
    wh8 = {}
    for gname in ("r", "z", "n"):
        w = wp.tile([P, EC, E], f8, tag=f"wh8{gname}")
        nc.sync.dma_start(w[:], D[f"wh8{gname}"].rearrange("(c p) f -> p c f", p=P))
        wh8[gname] = w
    wqk8 = wp.tile([P, EC, E], f8, tag="wqk8")
    nc.sync.dma_start(wqk8[:], D["wqk8"].rearrange("(c p) f -> p c f", p=P))
    # step-2 hidden weights + read weights share one 3-buffer tag: wvo's
    # buffer reuses wh16z's after the last step-2 z-gate matmul retires,
    # covered by the M'' fold matmuls that run first in the read phase.
    wh16 = {}
    for gname in ("z", "n"):
        w = wp.tile([P, EC, E], f16, tag="wbig", bufs=3)
        nc.sync.dma_start(w[:], D[f"wh{gname}"].rearrange("(c p) f -> p c f", p=P))
        wh16[gname] = w
    wqkr = wp.tile([P, EC, E], f16, tag="wbig", bufs=3)
    nc.sync.dma_start(wqkr[:], D["wqkr"].rearrange("(c p) f -> p c f", p=P))
    wvo = wp.tile([P, EC, E], f16, tag="wbig", bufs=3)
    nc.sync.dma_start(wvo[:], D["wvo"].rearrange("(c p) f -> p c f", p=P))
    if has_v:
        wvcol = wp.tile([P, EC, 1], f16, tag="wvcol")
        nc.sync.dma_start(wvcol[:], D["wvcol"].rearrange("(c p) f -> p c f", p=P))
    if has_rbo:
        nc.sync.dma_start(brep[:], D["brep"])

    sig_r = 1.0 / (aWgr * AU)
    sig_z = 1.0 / (aWgz * AU)

    # ---- LayerNorm split: stats (PE sums + row chain) / apply (broadcast)
    def emit_ln_stats(src):
        psx = psB.tile([1, 512], f32, tag="psB")
        for e in range(EC):
            nc.tensor.matmul(
                psx[:], lhsT=ones_c16[:], rhs=src[:, e, :],
                start=(e == 0), stop=(e == EC - 1),
            )
        mur = lnp.tile([1, 512], f16, tag="mur")
        nc.vector.tensor_scalar_mul(mur[:], psx[:], cinvE[:])
        psq = psB.tile([1, 512], f32, tag="psB")
        for e in range(EC):
            sq = lsp.tile([P, 512], f16, tag="ls")
            if e % 2 == 0:
                nc.scalar.square(sq[:], src[:, e, :])
            else:
                nc.vector.tensor_mul(sq[:], src[:, e, :], src[:, e, :])
            nc.tensor.matmul(
                psq[:], lhsT=ones_c16[:], rhs=sq[:],
                start=(e == 0), stop=(e == EC - 1),
            )
        mu2 = lnp.tile([1, 512], f16, tag="mu2")
        nc.vector.tensor_mul(mu2[:], mur[:], mur[:])
        varr = lnp.tile([1, 512], f16, tag="e2r")
        nc.vector.scalar_tensor_tensor(
            varr[:], psq[:], 1.0 / E, mu2[:], op0=Alu.mult, op1=Alu.subtract
        )
        nc.scalar.activation(varr[:], varr[:], AF.Sqrt, bias=eps1[:])
        rir = lnp.tile([1, 512], mybir.dt.float32r, tag="rir")
        with nc.allow_low_precision(reason="f32r rstd row for broadcast matmul"):
            nc.vector.reciprocal(rir[:], varr[:])
        msr = lnp.tile([1, 512], f16, tag="msr")
        nc.vector.tensor_mul(msr[:], mur[:], rir[:])
        return rir, msr

    def emit_ln_apply(src, st):
        rir, msr = st
        m16 = mnp.tile([P, EC, 512], f16, tag="mn16")
        m8 = mn8p.tile([P, EC, 512], f8, tag="mn8")
        psb = psA.tile([P, 512], f32, tag="psA")
        nc.tensor.matmul(psb[:], lhsT=ones_r32[:], rhs=rir[:])
        rstdb = lsp.tile([P, 512], f16, tag="ls")
        nc.scalar.copy(rstdb[:], psb[:])
        psb2 = psA.tile([P, 512], f32, tag="psA")
        nc.tensor.matmul(psb2[:], lhsT=ones_r16[:], rhs=msr[:])
        msb = lsp.tile([P, 512], f16, tag="ls")
        nc.scalar.copy(msb[:], psb2[:])
        for e in range(EC):
            t1 = lsp.tile([P, 512], f16, tag="ls")
            nc.vector.tensor_mul(t1[:], src[:, e, :], rstdb[:])
            if ln_affine:
                t2 = lsp.tile([P, 512], f16, tag="ls")
                nc.vector.tensor_sub(t2[:], t1[:], msb[:])
                nc.vector.tensor_scalar(
                    m16[:, e, :], t2[:], bcol(LNG, e), bcol(LNB, e),
                    op0=Alu.mult, op1=Alu.add,
                )
            else:
                nc.vector.tensor_sub(m16[:, e, :], t1[:], msb[:])
            if e % 2 == 0:
                nc.scalar.activation(m8[:, e, :], m16[:, e, :], AF.Copy, scale=AM)
            else:
                nc.vector.tensor_scalar_mul(m8[:, e, :], m16[:, e, :], cAM[:])
        return m16, m8

    # ---- per-group phases
    def emit_fold(m8g):
        """M'8 = fp8( memn8 @ Wqk8 + c_w ), feature-major [P, EC, 512]."""
        mp8 = mpp.tile([P, EC, 512], f8, tag="mp8")
        for c in range(EC):
            ps = psA.tile([P, 512], f32, tag="psA")
            for ep in range(3):
                nc.tensor.matmul(
                    ps[:], lhsT=wqk8[:, 2 * ep : 2 * ep + 2, c * 128 : (c + 1) * 128],
                    rhs=m8g[:, 2 * ep : 2 * ep + 2, :],
                    start=(ep == 0), stop=(ep == 2), perf_mode=DR,
                )
            if c % 2 == 0:
                nc.vector.tensor_scalar(
                    mp8[:, c, :], ps[:], cMPsc[:], bcol(CW, c),
                    op0=Alu.mult, op1=Alu.add,
                )
            else:
                nc.scalar.activation(
                    mp8[:, c, :], ps[:], AF.Identity,
                    scale=AMP / (AM * aWqk), bias=bcol(CW, c))
        return mp8

    def emit_scores(g, mp8, step0=False):
        """scores+softmax for the 4 batches of group g -> normalized an list."""
        an8s = []
        for bi in range(GB):
            b = g * GB + bi
            ps = psA.tile([P, L], f32, tag="psA")
            for ep in range(3):
                if step0:
                    lt = mp8[:, 2 * ep : 2 * ep + 2, :]
                else:
                    lt = mp8[:, 2 * ep : 2 * ep + 2, bi * 128 : (bi + 1) * 128]
                nc.tensor.matmul(
                    ps[:], lhsT=lt, rhs=z8f[b][:, 2 * ep : 2 * ep + 2, :],
                    start=(ep == 0), stop=(ep == 2), perf_mode=DR,
                )
            aexp = sxp.tile([P, L], f16, tag="aexp")
            rsum = smp.tile([P, 1], f32, tag="rsum")
            nc.scalar.activation(
                aexp[:], ps[:], AF.Exp, scale=1.0 / (AMP * AZ), accum_out=rsum[:]
            )
            rinv = smp.tile([P, 1], f32, tag="rinv")
            nc.vector.reciprocal(rinv[:], rsum[:])
            an16 = anp.tile([P, L], f16, tag="an16")
            nc.vector.tensor_scalar_mul(an16[:], aexp[:], rinv[:])
            an8s.append(an16)
        return an8s

    def emit_transposes(an8s):
        att8 = []
        for bi in range(GB):
            at = atp.tile([P, 4, P], f8, tag="att8")
            pt4 = psT.tile([P, 4, P], f16, tag="psT")
            for t4 in range(4):
                nc.tensor.transpose(
                    pt4[:, t4, :], an8s[bi][:, t4 * 128 : (t4 + 1) * 128], idy16[:]
                )
            if bi % 2 == 0:
                nc.vector.tensor_scalar_mul(at[:], pt4[:], cAA[:])
            else:
                nc.scalar.activation(at[:], pt4[:], AF.Copy, scale=AA)
            att8.append(at)
        return att8

    def emit_ut(g, att8):
        """ut8 = fp8( (A@z).T ) feature-major [P, EC, 512]."""
        ut8 = utp.tile([P, EC, 512], f8, tag="ut8")
        for c in range(EC):
            ps = psA.tile([P, 512], f32, tag="psA")
            for bi in range(GB):
                b = g * GB + bi
                for tp in range(2):
                    nc.tensor.matmul(
                        ps[:, bi * 128 : (bi + 1) * 128],
                        lhsT=z8t[b][:, 2 * tp : 2 * tp + 2, c * 128 : (c + 1) * 128],
                        rhs=att8[bi][:, 2 * tp : 2 * tp + 2, :],
                        start=(tp == 0), stop=(tp == 1), perf_mode=DR,
                    )
            if c % 2 == 0:
                nc.vector.tensor_scalar_mul(ut8[:, c, :], ps[:], cUsc[:])
            else:
                nc.scalar.activation(
                    ut8[:, c, :], ps[:], AF.Copy, scale=AU / (AA * AZ)
                )
        return ut8

    def emit_gates(g, ut8, step, h16=None, chunks=None):
        """GRU gates chunk-major; writes h' to a new mem16 tile.

        step==0: hidden-side terms come from host (gh0t via block-repeat
        matmul for r/z, hn0rep as a direct SBUF operand for n).
        step>=1: hidden side contracts memn8 (fp8 DR) or memn16 (f16)
        per FP8_GH[step].
        """
        if h16 is None:
            h16 = mp.tile([P, EC, 512], f16, tag="h16")
        fp8g = FP8_GH.get(step, ())
        mn16g = mn16_0 if step == 0 else memn16[g]
        for c in (chunks if chunks is not None else range(EC)):
            cs = slice(c * 128, (c + 1) * 128)

            def gi_mms(psum, gname, stop_last):
                for ep in range(3):
                    nc.tensor.matmul(
                        psum[:], lhsT=wg8[gname][:, 2 * ep : 2 * ep + 2, cs],
                        rhs=ut8[:, 2 * ep : 2 * ep + 2, :],
                        start=(ep == 0), stop=(stop_last and ep == 2),
                        perf_mode=DR,
                    )

            def gh_mms(psum, gname):
                # hidden-side contraction for steps >= 1
                if gname in fp8g:
                    for ep in range(3):
                        nc.tensor.matmul(
                            psum[:], lhsT=wh8[gname][:, 2 * ep : 2 * ep + 2, cs],
                            rhs=memn8[g][:, 2 * ep : 2 * ep + 2, :],
                            start=False, stop=(ep == 2), perf_mode=DR,
                        )
                else:
                    for e in range(EC):
                        nc.tensor.matmul(
                            psum[:], lhsT=wh16[gname][:, e, cs],
                            rhs=mn16g[:, e, :],
                            start=False, stop=(e == EC - 1),
                        )

            ps_r = psA.tile([P, 512], f32, tag="psA")
            gi_mms(ps_r, "r", stop_last=False)
            if step == 0:
                nc.tensor.matmul(
                    ps_r[:], lhsT=gh0t["r"][:, cs], rhs=idyrep[:],
                    start=False, stop=True,
                )
            else:
                gh_mms(ps_r, "r")
            r16 = gsp.tile([P, 512], f16, tag="gs")
            nc.scalar.activation(
                r16[:], ps_r[:], AF.Sigmoid, scale=sig_r, bias=bcol(SGR, c)
            )

            ps_z = psA.tile([P, 512], f32, tag="psA")
            gi_mms(ps_z, "z", stop_last=False)
            if step == 0:
                nc.tensor.matmul(
                    ps_z[:], lhsT=gh0t["z"][:, cs], rhs=idyrep[:],
                    start=False, stop=True,
                )
            else:
                gh_mms(ps_z, "z")
            zt16 = gsp.tile([P, 512], f16, tag="gs")
            nc.scalar.activation(
                zt16[:], ps_z[:], AF.Sigmoid, scale=sig_z, bias=bcol(SGZ, c)
            )

            ps_i = psA.tile([P, 512], f32, tag="psA")
            gi_mms(ps_i, "n", stop_last=True)
            if step == 0:
                hn16 = hn0rep[:, c, :]
            else:
                ps_h = psA.tile([P, 512], f32, tag="psA")
                if "n" in fp8g:
                    for ep in range(3):
                        nc.tensor.matmul(
                            ps_h[:], lhsT=wh8["n"][:, 2 * ep : 2 * ep + 2, cs],
                            rhs=memn8[g][:, 2 * ep : 2 * ep + 2, :],
                            start=(ep == 0), stop=(ep == 2), perf_mode=DR,
                        )
                else:
                    for e in range(EC):
                        nc.tensor.matmul(
                            ps_h[:], lhsT=wh16["n"][:, e, cs], rhs=mn16g[:, e, :],
                            start=(e == 0), stop=(e == EC - 1),
                        )
                hn16 = gsp.tile([P, 512], f16, tag="gs")
                nc.vector.tensor_scalar(
                    hn16[:], ps_h[:], b1col(COL_SCN), bcol(BHN, c),
                    op0=Alu.mult, op1=Alu.add,
                )
            t16 = gsp.tile([P, 512], f16, tag="gs")
            nc.vector.tensor_mul(t16[:], r16[:], hn16[:])
            # arg = gi_n/(aWgn*AU) + r*hn; the +bgn bias rides the tanh
            arg16 = gsp.tile([P, 512], f16, tag="gs")
            nc.vector.scalar_tensor_tensor(
                arg16[:], ps_i[:], b1col(COL_SCN), t16[:],
                op0=Alu.mult, op1=Alu.add,
            )
            n16 = gsp.tile([P, 512], f16, tag="gs")
            nc.scalar.activation(n16[:], arg16[:], AF.Tanh, bias=bcol(BGN, c))
            d16 = gsp.tile([P, 512], f16, tag="gs")
            nc.vector.tensor_sub(d16[:], mn16g[:, c, :], n16[:])
            t2 = gsp.tile([P, 512], f16, tag="gs")
            nc.vector.tensor_mul(t2[:], zt16[:], d16[:])
            nc.vector.tensor_add(h16[:, c, :], n16[:], t2[:])
        return h16

    # ---- step 0 (batch-invariant memory; fold + hidden side from host)
    memn16 = [None, None]
    memn8 = [None, None]
    an0 = emit_scores(0, mp0_8, step0=True)
    att0 = emit_transposes(an0)
    ut0 = emit_ut(0, att0)
    an1 = emit_scores(1, mp0_8, step0=True)
    att1 = emit_transposes(an1)
    h0 = emit_gates(0, ut0, 0, chunks=range(0, 2))
    ut1 = emit_ut(1, att1)
    h0 = emit_gates(0, ut0, 0, h16=h0, chunks=range(2, EC))
    s0 = emit_ln_stats(h0)
    h1 = emit_gates(1, ut1, 0, chunks=range(0, 3))
    memn16[0], memn8[0] = emit_ln_apply(h0, s0)
    h1 = emit_gates(1, ut1, 0, h16=h1, chunks=range(3, EC))
    pend1 = (h1, emit_ln_stats(h1))

    # ---- steps 1..T-1, LN-apply for group 1 deferred into the next step
    for step in range(1, T):
        mp8_0 = emit_fold(memn8[0])
        an0 = emit_scores(0, mp8_0)
        att0 = emit_transposes(an0)
        if pend1 is not None:
            hp, sp = pend1
            memn16[1], memn8[1] = emit_ln_apply(hp, sp)
            pend1 = None
        ut0 = emit_ut(0, att0)
        mp8_1 = emit_fold(memn8[1])
        an1 = emit_scores(1, mp8_1)
        att1 = emit_transposes(an1)
        h0 = emit_gates(0, ut0, step, chunks=range(0, 2))
        ut1 = emit_ut(1, att1)
        h0 = emit_gates(0, ut0, step, h16=h0, chunks=range(2, EC))
        if step < T - 1:
            s0 = emit_ln_stats(h0)
            h1 = emit_gates(1, ut1, step, chunks=range(0, 3))
            memn16[0], memn8[0] = emit_ln_apply(h0, s0)
            h1 = emit_gates(1, ut1, step, h16=h1, chunks=range(3, EC))
            pend1 = (h1, emit_ln_stats(h1))
        else:
            h1 = emit_gates(1, ut1, step)
            mem16 = [h0, h1]

    # ---- read attention: M'' (fm, fp8 at AMP), v, V'' (slot-major) per group
    m28_g, v16_g, vpp16_g = [], [], []
    for g in range(NG):
        m28 = mpp.tile([P, EC, 512], f8, tag="mp8")
        for c in range(EC):
            ps = psA.tile([P, 512], f32, tag="psA")
            for e in range(EC):
                nc.tensor.matmul(
                    ps[:], lhsT=wqkr[:, e, c * 128 : (c + 1) * 128],
                    rhs=mem16[g][:, e, :],
                    start=(e == 0), stop=(e == EC - 1),
                )
            if c % 2 == 0:
                nc.vector.tensor_scalar(
                    m28[:, c, :], ps[:], AMP, bcol(CR, c),
                    op0=Alu.mult, op1=Alu.add,
                )
            else:
                nc.scalar.activation(
                    m28[:, c, :], ps[:], AF.Identity, scale=AMP, bias=bcol(CR, c))
        m28_g.append(m28)
        if has_v:
            v16 = rvp.tile([P, GB], f32, tag="v16")
            for bi in range(GB):
                psv = psT.tile([P, 1], f32, tag="psT")
                for e in range(EC):
                    nc.tensor.matmul(
                        psv[:], lhsT=mem16[g][:, e, bi * 128 : (bi + 1) * 128],
                        rhs=wvcol[:, e, :],
                        start=(e == 0), stop=(e == EC - 1),
                    )
                nc.scalar.activation(
                    v16[:, bi : bi + 1], psv[:], AF.Identity, bias=b1col(COL_VC)
                )
            v16_g.append(v16)
        else:
            v16_g.append(None)
        vpp = rvp.tile([P, GB, E], f16, tag="vpp16")
        for bi in range(GB):
            for n0, nw in ((0, 512), (512, 256)):
                ps = psA.tile([P, nw], f32, tag="psA")
                for e in range(EC):
                    nc.tensor.matmul(
                        ps[:], lhsT=mem16[g][:, e, bi * 128 : (bi + 1) * 128],
                        rhs=wvo[:, e, n0 : n0 + nw],
                        start=(e == 0), stop=(not has_rbo and e == EC - 1),
                    )
                # rank-1 + 1 x rbo_eff: after the deferred 1/colsum scaling the
                # per-token bias comes out exactly as rbo_eff.
                if has_rbo:
                    nc.tensor.matmul(
                        ps[:], lhsT=ones_r16[:], rhs=brep[:, n0 : n0 + nw],
                        start=False, stop=True,
                    )
                if bi % 2 == 0:
                    nc.scalar.copy(vpp[:, bi, n0 : n0 + nw], ps[:])
                else:
                    nc.vector.tensor_copy(vpp[:, bi, n0 : n0 + nw], ps[:])
        vpp16_g.append(vpp)

    # ---- read attention per batch, software-pipelined by one batch so the
    # PE's AV matmuls of batch i overlap the exp/recip of batch i+1.
    def emit_read_scores(idx):
        g, bi = idx // GB, idx % GB
        b = idx
        ps_s = psA.tile([P, L], f32, tag="psA")
        if FP8_READ_SC:
            for ep in range(3):
                nc.tensor.matmul(
                    ps_s[:],
                    lhsT=m28_g[g][:, 2 * ep : 2 * ep + 2, bi * 128 : (bi + 1) * 128],
                    rhs=z8f[b][:, 2 * ep : 2 * ep + 2, :],
                    start=(ep == 0), stop=(ep == 2), perf_mode=DR,
                )
        else:
            for e in range(EC):
                nc.tensor.matmul(
                    ps_s[:],
                    lhsT=m28_g[g][:, e, bi * 128 : (bi + 1) * 128],
                    rhs=z8f[b][:, e, :],
                    start=(e == 0), stop=(e == EC - 1),
                )
        eS = sxp.tile([P, L], f16, tag="eS")
        if has_v:
            nc.scalar.activation(
                eS[:], ps_s[:], AF.Exp, scale=1.0 / (AMP * AZ),
                bias=v16_g[g][:, bi : bi + 1],
            )
        else:
            nc.scalar.activation(eS[:], ps_s[:], AF.Exp, scale=1.0 / (AMP * AZ))
        return eS

    def emit_read_av(idx, eS):
        g, bi = idx // GB, idx % GB
        b = idx
        rc4ps = psT.tile([P, 4], f32, tag="psT")
        for t4 in range(4):
            nc.tensor.matmul(
                rc4ps[:, t4 : t4 + 1],
                lhsT=eS[:, t4 * 128 : (t4 + 1) * 128],
                rhs=ones_c16[:],
            )
        rc4 = smp.tile([P, 4], f32, tag="rc4")
        nc.vector.reciprocal(rc4[:], rc4ps[:])
        for t4 in range(4):
            osb = osp.tile([P, E], f16, tag="osb")
            for n0, nw in ((0, 512), (512, 256)):
                if t4 % 2 == 0:
                    ps = psB.tile([P, nw], f32, tag="psB")
                else:
                    ps = psA.tile([P, nw], f32, tag="psA")
                nc.tensor.matmul(
                    ps[:],
                    lhsT=eS[:, t4 * 128 : (t4 + 1) * 128],
                    rhs=vpp16_g[g][:, bi, n0 : n0 + nw],
                )
                if n0 == 0:
                    nc.vector.tensor_scalar_mul(
                        osb[:, n0 : n0 + nw], ps[:], rc4[:, t4 : t4 + 1]
                    )
                else:
                    nc.scalar.activation(
                        osb[:, n0 : n0 + nw], ps[:], AF.Copy,
                        scale=rc4[:, t4 : t4 + 1],
                    )
            if t4 % 2 == 0:
                nc.sync.dma_start(D["out"][b, t4 * 128 : (t4 + 1) * 128, :], osb[:])
            else:
                nc.scalar.dma_start(D["out"][b, t4 * 128 : (t4 + 1) * 128, :], osb[:])

    eS_prev = None
    for idx in range(NB):
        eS_cur = emit_read_scores(idx)
        if eS_prev is not None:
            emit_read_av(idx - 1, eS_prev)
        eS_prev = eS_cur
    emit_read_av(NB - 1, eS_prev)


def _build(sc):
    key = tuple(sorted(sc.items()))
    if key in _CACHE:
        return _CACHE[key]
    nc = bacc.Bacc(
        "TRN2", target_bir_lowering=False, debug=False, enable_asserts=False
    )
    D = {}
    D["z8f"] = nc.dram_tensor("z8f", [NB, E, L], f8, kind="ExternalInput").ap()
    D["z8t"] = nc.dram_tensor("z8t", [NB, L, E], f8, kind="ExternalInput").ap()
    D["wqk8"] = nc.dram_tensor("wqk8", [E, E], f8, kind="ExternalInput").ap()
    for gname in ("r", "z", "n"):
        D[f"wg8{gname}"] = nc.dram_tensor(
            f"wg8{gname}", [E, E], f8, kind="ExternalInput").ap()
        D[f"wh8{gname}"] = nc.dram_tensor(
            f"wh8{gname}", [E, E], f8, kind="ExternalInput").ap()
    for gname in ("z", "n"):
        D[f"wh{gname}"] = nc.dram_tensor(
            f"wh{gname}", [E, E], f16, kind="ExternalInput").ap()
    D["wqkr"] = nc.dram_tensor("wqkr", [E, E], f16, kind="ExternalInput").ap()
    D["wvo"] = nc.dram_tensor("wvo", [E, E], f16, kind="ExternalInput").ap()
    D["wvcol"] = nc.dram_tensor("wvcol", [E, 1], f16, kind="ExternalInput").ap()
    D["mn0_16"] = nc.dram_tensor("mn0_16", [E, S], f16, kind="ExternalInput").ap()
    D["mp0_8"] = nc.dram_tensor("mp0_8", [E, S], f8, kind="ExternalInput").ap()
    D["gh0r"] = nc.dram_tensor("gh0r", [S, E], f16, kind="ExternalInput").ap()
    D["gh0z"] = nc.dram_tensor("gh0z", [S, E], f16, kind="ExternalInput").ap()
    D["hn0"] = nc.dram_tensor("hn0", [E, 512], f16, kind="ExternalInput").ap()
    D["bias"] = nc.dram_tensor("bias", [P, NCOLS], f32, kind="ExternalInput").ap()
    D["brep"] = nc.dram_tensor("brep", [1, E], f16, kind="ExternalInput").ap()
    D["out"] = nc.dram_tensor("out", [NB, L, E], f16, kind="ExternalOutput").ap()
    with tile.TileContext(nc) as tc:
        with ExitStack() as ctx:
            _emit(nc, tc, ctx, D, sc)
    nc.compile()
    _CACHE[key] = nc
    return nc


def _pow2_alpha(x, target=120.0):
    m = float(np.abs(x).max())
    if m < 1e-30:
        return 1.0
    return float(2.0 ** np.floor(np.log2(target / m)))


def _q8(x, alpha):
    xs = np.asarray(x, np.float32) * alpha
    return np.ascontiguousarray(
        np.clip(xs, -224.0, 224.0).astype(ml_dtypes.float8_e4m3)
    )


def _t16(a):
    return np.ascontiguousarray(np.asarray(a, np.float32).astype(np.float16))


def _host_prep(inp):
    s = 1.0 / np.sqrt(float(E))
    W = {k: np.asarray(v, np.float64) for k, v in inp.items()}

    # --- folds (f64)
    Wqk_w = (W["w_wq"].T @ W["w_wk"]) * s
    c_w = (W["w_bq"] @ W["w_wk"]) * s
    wo, wv = W["w_wo"], W["w_wv"]
    wih, whh = W["gru_wih"], W["gru_whh"]
    bih, bhh = W["gru_bih"], W["gru_bhh"]
    bo, bv = W["w_bo"], W["w_bv"]
    Wg, bgc, Whg, bhgc = {}, {}, {}, {}
    for i, gname in enumerate(("r", "z", "n")):
        wg = wih[i * E:(i + 1) * E]
        Wg[gname] = wg @ wo @ wv
        bgc[gname] = bv @ (wg @ wo).T + bo @ wg.T + bih[i * E:(i + 1) * E]
        Whg[gname] = whh[i * E:(i + 1) * E]
        bhgc[gname] = bhh[i * E:(i + 1) * E]
    Wqk_r = (W["r_wk"].T @ W["r_wq"]) * s
    c_r = (W["r_bk"] @ W["r_wq"]) * s
    w_v = (W["r_wk"].T @ W["r_bq"]) * s
    v_const = float((W["r_bk"] @ W["r_bq"]) * s)
    Wvo_r = W["r_wv"].T @ W["r_wo"].T
    c_vo = W["r_bv"] @ W["r_wo"].T
    rbo_eff = W["r_bo"] + c_vo

    # --- scales
    sc = {
        "ln_affine": bool(
            np.abs(np.asarray(W["ln_g"]) - 1.0).max() > 0
            or np.abs(np.asarray(W["ln_b"])).max() > 0
        ),
        "has_v": bool(np.abs(w_v).max() > 0 or v_const != 0.0),
        "has_rbo": bool(np.abs(rbo_eff).max() > 0),
        "aWqk": _pow2_alpha(Wqk_w),
        "aWgr": _pow2_alpha(np.concatenate([Wg["r"].ravel(), Whg["r"].ravel()])),
        "aWgz": _pow2_alpha(np.concatenate([Wg["z"].ravel(), Whg["z"].ravel()])),
        "aWgn": _pow2_alpha(np.concatenate([Wg["n"].ravel(), Whg["n"].ravel()])),
    }
    for gname, a in (("r", sc["aWgr"]), ("z", sc["aWgz"]), ("n", sc["aWgn"])):
        assert np.abs(Whg[gname]).max() * a * AU < 60000.0

    shared = {}
    # weights stored [in, out] so lhsT tiles contract over the partition dim
    shared["wqk8"] = _q8(Wqk_w, sc["aWqk"])
    for gname in ("r", "z", "n"):
        a = sc[f"aWg{gname}"]
        shared[f"wg8{gname}"] = _q8(Wg[gname].T, a)     # [in, out]
        shared[f"wh8{gname}"] = _q8(Whg[gname].T, a)    # [in, out] fp8
    for gname in ("z", "n"):
        a = sc[f"aWg{gname}"]
        shared[f"wh{gname}"] = _t16(Whg[gname].T * (a * AU))
    shared["wqkr"] = _t16(Wqk_r)
    shared["wvo"] = _t16(Wvo_r)
    shared["wvcol"] = _t16(w_v.reshape(E, 1))

    # initial memn = LN(slots) (batch-invariant), feature-major [E, S]
    slots = np.asarray(W["slots"], np.float64)[0]       # [S, E]
    g64 = W["ln_g"]
    b64 = W["ln_b"]
    mu = slots.mean(-1, keepdims=True)
    var = ((slots - mu) ** 2).mean(-1, keepdims=True)
    mn0 = (slots - mu) / np.sqrt(var + LN_EPS) * g64 + b64   # [S, E]
    shared["mn0_16"] = _t16(mn0.T)

    # step-0 host precomputes: score fold and GRU hidden-side projections
    shared["mp0_8"] = _q8((mn0 @ Wqk_w + c_w[None, :]).T, AMP)     # [E, S]
    for gname in ("r", "z"):
        a = sc[f"aWg{gname}"]
        g0 = (mn0 @ Whg[gname].T) * (a * AU)                       # [S, E]
        assert np.abs(g0).max() < 60000.0
        shared[f"gh0{gname}"] = _t16(g0)
    hn0 = (mn0 @ Whg["n"].T) + bhgc["n"][None, :]                  # [S, E]
    assert np.abs(hn0).max() < 60000.0
    shared["hn0"] = np.ascontiguousarray(np.tile(_t16(hn0).T, (1, GB)))  # [E,512]

    def col6(vec):
        return np.asarray(vec, np.float32).reshape(EC, P).T

    cols = [
        col6(W["ln_g"]), col6(W["ln_b"]),
        col6(bgc["r"] + bhgc["r"]), col6(bgc["z"] + bhgc["z"]),
        col6(bgc["n"]), col6(bhgc["n"]),
        col6(c_r * AMP), col6(c_w * AMP),
    ]
    btab = np.concatenate(cols, axis=1)
    vc = np.full((P, 1), v_const, np.float32)
    scn = np.full((P, 1), 1.0 / (sc["aWgn"] * AU), np.float32)
    shared["bias"] = np.ascontiguousarray(
        np.concatenate([btab, vc, scn], axis=1), np.float32)
    shared["brep"] = np.ascontiguousarray(
        np.asarray(rbo_eff, np.float32)[None, :].astype(np.float16))

    # --- per-core z arrays
    z = np.asarray(inp["z"], np.float32)                # [B, L, E]
    zT = np.swapaxes(z, 1, 2)                           # [B, E, L]
    z8f_all = _q8(zT, AZ)
    z8t_all = _q8(z, AZ)
    in_maps = []
    for c in range(NCORE):
        m = dict(shared)
        sl = slice(c * NB, (c + 1) * NB)
        m["z8f"] = np.ascontiguousarray(z8f_all[sl])
        m["z8t"] = np.ascontiguousarray(z8t_all[sl])
        in_maps.append(m)
    return in_maps, sc


def kernel(**inputs):
    in_maps, sc = _host_prep(inputs)
    nc = _build(sc)
    res = bass_utils.run_bass_kernel_spmd(nc, in_maps, core_ids=list(range(NCORE)))
    out = np.concatenate([res.results[c]["out"] for c in range(NCORE)], axis=0)
    return out.astype(np.float32)
